# revision 1
# baseline (speedup 1.0000x reference)
"""Trainium2 Bass kernel for DependencyGNN (2-layer GCN + global mean pool).

Distribution: nodes sharded contiguously across 8 cores (25000 each, padded to
25088 = 196*128). Edges assigned to the owner of their dst node, grouped into
128-node dst windows, padded per-window to a multiple of 128 (window tile
counts equalized across cores so all cores run one SPMD program).

Per core:
  stage 1: h1_lin = x_shard @ W1            (PE, xT supplied feature-major)
  AllGather h1_lin -> h1full (replicated)
  layer 1: gather h1full[src] per edge tile; aggregate via PE matmul with a
           one-hot*norm selection matrix; + self-loop + b1; relu -> h1
  AllGather h1 -> h1rfull
  layer 2: same aggregation on h1rfull -> agg2pre (pre-W2, since W2/b2/pooling
           commute past the linear aggregation)
Host epilogue: segment-mean-pool agg2pre over graphs, @W2 + b2.
"""
import numpy as np

N_NODES = 200000
N_EDGES = 400000
IN_CH, HID_CH, OUT_CH = 768, 256, 256
NUM_GRAPHS = 8000
N_CORES = 8
P = 128
NPC = N_NODES // N_CORES            # 25000 nodes per core
NT = (NPC + P - 1) // P             # 196 m-tiles per core
NPC_PAD = NT * P                    # 25088
K_TILES = IN_CH // P                # 6


def _prep(x, W1, b1, W2, b2, edge_index, batch):
    x = np.asarray(x, dtype=np.float32)
    W1 = np.asarray(W1, dtype=np.float32)
    b1 = np.asarray(b1, dtype=np.float32)
    W2 = np.asarray(W2, dtype=np.float32)
    b2 = np.asarray(b2, dtype=np.float32)
    src = np.asarray(edge_index[0], dtype=np.int64)
    dst = np.asarray(edge_index[1], dtype=np.int64)
    batch = np.asarray(batch, dtype=np.int64)

    # degree (dst-side, incl self-loop), symmetric norm
    deg = np.bincount(dst, minlength=N_NODES).astype(np.float64) + 1.0
    dinv = (1.0 / np.sqrt(deg)).astype(np.float32)
    enorm = dinv[src] * dinv[dst]
    dinv2 = (dinv * dinv).astype(np.float32)

    # global row index in the AG-concatenated (padded) table
    def grow(n):
        return (n // NPC) * NPC_PAD + (n % NPC)

    owner = dst // NPC
    win = (dst % NPC) // P
    # per (core, window) edge counts -> equalized tile counts
    cnt = np.zeros((N_CORES, NT), dtype=np.int64)
    np.add.at(cnt, (owner, win), 1)
    tw = np.maximum((cnt.max(axis=0) + P - 1) // P, 1).astype(np.int64)  # [NT]
    ET = int(tw.sum())
    col_base = np.zeros(NT, dtype=np.int64)
    col_base[1:] = np.cumsum(tw)[:-1]
    win_of_col = np.repeat(np.arange(NT), tw)            # [ET]

    gsrc = np.zeros((N_CORES, P, ET), dtype=np.int32)
    gdst = np.full((N_CORES, P, ET), -1.0, dtype=np.float32)
    gnrm = np.zeros((N_CORES, P, ET), dtype=np.float32)

    order = np.lexsort((dst, win, owner))
    so, sw = owner[order], win[order]
    ssrc, sdst, snorm = src[order], dst[order], enorm[order]
    # position within (core, window) group
    grp = so * NT + sw
    first = np.zeros(N_CORES * NT, dtype=np.int64)
    np.add.at(first, grp, 1)
    starts = np.zeros(N_CORES * NT, dtype=np.int64)
    starts[1:] = np.cumsum(first)[:-1]
    pos = np.arange(len(order)) - starts[grp]
    col = col_base[sw] + pos // P
    row = pos % P
    gsrc[so, row, col] = grow(ssrc).astype(np.int32)
    gdst[so, row, col] = (sdst % NPC - sw * P).astype(np.float32)
    gnrm[so, row, col] = snorm

    # per-core tensors
    in_maps = []
    xpad = np.zeros((NPC_PAD, IN_CH), dtype=np.float32)
    d2 = np.zeros((P, NT), dtype=np.float32)
    for c in range(N_CORES):
        lo = c * NPC
        xpad[:NPC] = x[lo:lo + NPC]
        xt = np.ascontiguousarray(xpad.T)                 # [768, 25088]
        d2[:] = 0.0
        dv = dinv2[lo:lo + NPC]
        d2c = np.zeros(NPC_PAD, dtype=np.float32)
        d2c[:NPC] = dv
        d2 = np.ascontiguousarray(d2c.reshape(NT, P).T)   # [128, NT]
        in_maps.append({
            "xt": xt.copy(),
            "w1": W1,
            "gsrc": np.ascontiguousarray(gsrc[c]),
            "gdst": np.ascontiguousarray(gdst[c]),
            "gnrm": np.ascontiguousarray(gnrm[c]),
            "dinv2": d2.copy(),
            "b1b": np.tile(b1[None, :], (P, 1)),
            "iota": np.tile(np.arange(P, dtype=np.float32)[None, :], (P, 1)),
        })

    meta = {"ET": ET, "tw": tw.tolist()}
    counts = np.bincount(batch, minlength=NUM_GRAPHS).astype(np.int64)
    seg_starts = np.minimum(
        np.searchsorted(batch, np.arange(NUM_GRAPHS)), N_NODES - 1
    )
    aux = {"W2": W2, "b2": b2, "counts": counts, "seg_starts": seg_starts}
    return meta, in_maps, aux


def _build(meta):
    import concourse.bass as bass
    import concourse.bacc as bacc
    import concourse.mybir as mybir
    import concourse.tile as tile

    ET = meta["ET"]
    tw = meta["tw"]
    f32 = mybir.dt.float32

    nc = bacc.Bacc()
    xt = nc.declare_dram_parameter("xt", [IN_CH, NPC_PAD], f32, isOutput=False)
    w1 = nc.declare_dram_parameter("w1", [IN_CH, HID_CH], f32, isOutput=False)
    gsrc = nc.declare_dram_parameter("gsrc", [P, ET], mybir.dt.int32, isOutput=False)
    gdst = nc.declare_dram_parameter("gdst", [P, ET], f32, isOutput=False)
    gnrm = nc.declare_dram_parameter("gnrm", [P, ET], f32, isOutput=False)
    dinv2 = nc.declare_dram_parameter("dinv2", [P, NT], f32, isOutput=False)
    b1b = nc.declare_dram_parameter("b1b", [P, HID_CH], f32, isOutput=False)
    iota = nc.declare_dram_parameter("iota", [P, P], f32, isOutput=False)
    out_pre = nc.declare_dram_parameter("out_pre", [NPC_PAD, HID_CH], f32, isOutput=True)

    h1l = nc.dram_tensor("h1l", [NPC_PAD, HID_CH], f32)                       # ag1 in
    h1full = nc.dram_tensor("h1full", [NPC_PAD * N_CORES, HID_CH], f32, addr_space="Shared")
    h1own = nc.dram_tensor("h1own", [NPC_PAD, HID_CH], f32)                   # ag2 in
    h1rfull = nc.dram_tensor("h1rfull", [NPC_PAD * N_CORES, HID_CH], f32, addr_space="Shared")

    # ---- stage 1: h1l = x @ W1 ----
    with tile.TileContext(nc) as tc:
        with (
            tc.tile_pool(name="s1", bufs=3) as sbuf,
            tc.tile_pool(name="s1c", bufs=1) as cbuf,
            tc.tile_pool(name="p1", bufs=2, space="PSUM") as psum,
        ):
            w1_t = cbuf.tile([P, K_TILES, HID_CH], f32)
            nc.sync.dma_start(out=w1_t[:], in_=w1[:].rearrange("(a k) n -> k a n", k=P))
            for m in range(NT):
                xt_t = sbuf.tile([P, K_TILES, P], f32, tag="xt")
                nc.sync.dma_start(
                    out=xt_t[:],
                    in_=xt[:, m * P:(m + 1) * P].rearrange("(a k) m -> k a m", k=P),
                )
                acc = psum.tile([P, HID_CH], f32, tag="acc")
                for k in range(K_TILES):
                    nc.tensor.matmul(
                        acc[:], lhsT=xt_t[:, k, :], rhs=w1_t[:, k, :],
                        start=(k == 0), stop=(k == K_TILES - 1),
                    )
                h = sbuf.tile([P, HID_CH], f32, tag="h")
                nc.vector.tensor_copy(out=h[:], in_=acc[:])
                nc.sync.dma_start(out=h1l[m * P:(m + 1) * P, :], in_=h[:])

    cc_sem = nc.semaphore("cc_sem").__enter__()
    nc.gpsimd.collective_compute(
        "AllGather", mybir.AluOpType.bypass,
        ins=[h1l[:]], outs=[h1full[:]],
        replica_groups=[list(range(N_CORES))],
    ).then_inc(cc_sem, 1)
    nc.gpsimd.wait_ge(cc_sem, 1)

    # ---- aggregation layer (shared builder) ----
    def agg_layer(table, local_in, dest, relu_bias):
        with tile.TileContext(nc) as tc:
            with (
                tc.tile_pool(name="sa", bufs=4) as sbuf,
                tc.tile_pool(name="sac", bufs=1) as cbuf,
                tc.tile_pool(name="pa", bufs=2, space="PSUM") as psum,
            ):
                gsrc_t = cbuf.tile([P, ET], mybir.dt.int32)
                gdst_t = cbuf.tile([P, ET], f32)
                gnrm_t = cbuf.tile([P, ET], f32)
                iota_t = cbuf.tile([P, P], f32)
                d2_t = cbuf.tile([P, NT], f32)
                b1_t = cbuf.tile([P, HID_CH], f32)
                nc.sync.dma_start(out=gsrc_t[:], in_=gsrc[:])
                nc.sync.dma_start(out=gdst_t[:], in_=gdst[:])
                nc.sync.dma_start(out=gnrm_t[:], in_=gnrm[:])
                nc.sync.dma_start(out=iota_t[:], in_=iota[:])
                nc.sync.dma_start(out=d2_t[:], in_=dinv2[:])
                nc.sync.dma_start(out=b1_t[:], in_=b1b[:])
                col = 0
                for w in range(NT):
                    acc = psum.tile([P, HID_CH], f32, tag="acc")
                    for t in range(tw[w]):
                        msg = sbuf.tile([P, HID_CH], f32, tag="msg")
                        nc.gpsimd.indirect_dma_start(
                            out=msg[:], out_offset=None, in_=table[:],
                            in_offset=bass.IndirectOffsetOnAxis(
                                ap=gsrc_t[:, col:col + 1], axis=0),
                        )
                        pt = sbuf.tile([P, P], f32, tag="pt")
                        nc.vector.tensor_tensor(
                            out=pt[:], in0=gdst_t[:, col:col + 1].to_broadcast([P, P]),
                            in1=iota_t[:], op=mybir.AluOpType.is_equal,
                        )
                        nc.vector.tensor_tensor(
                            out=pt[:], in0=pt[:],
                            in1=gnrm_t[:, col:col + 1].to_broadcast([P, P]),
                            op=mybir.AluOpType.mult,
                        )
                        nc.tensor.matmul(
                            acc[:], lhsT=pt[:], rhs=msg[:],
                            start=(t == 0), stop=(t == tw[w] - 1),
                        )
                        col += 1
                    # epilogue: + dinv2 * local_in  (+ b1, relu for layer 1)
                    loc = sbuf.tile([P, HID_CH], f32, tag="loc")
                    nc.sync.dma_start(out=loc[:], in_=local_in[w * P:(w + 1) * P, :])
                    tmp = sbuf.tile([P, HID_CH], f32, tag="tmp")
                    nc.vector.tensor_tensor(
                        out=tmp[:], in0=loc[:],
                        in1=d2_t[:, w:w + 1].to_broadcast([P, HID_CH]),
                        op=mybir.AluOpType.mult,
                    )
                    nc.vector.tensor_tensor(
                        out=tmp[:], in0=tmp[:], in1=acc[:], op=mybir.AluOpType.add,
                    )
                    outt = sbuf.tile([P, HID_CH], f32, tag="outt")
                    if relu_bias:
                        nc.vector.tensor_tensor(
                            out=tmp[:], in0=tmp[:], in1=b1_t[:], op=mybir.AluOpType.add,
                        )
                        nc.scalar.activation(
                            out=outt[:], in_=tmp[:],
                            func=mybir.ActivationFunctionType.Relu,
                        )
                    else:
                        nc.vector.tensor_copy(out=outt[:], in_=tmp[:])
                    nc.sync.dma_start(out=dest[w * P:(w + 1) * P, :], in_=outt[:])

    agg_layer(h1full, h1l, h1own, relu_bias=True)

    nc.gpsimd.collective_compute(
        "AllGather", mybir.AluOpType.bypass,
        ins=[h1own[:]], outs=[h1rfull[:]],
        replica_groups=[list(range(N_CORES))],
    ).then_inc(cc_sem, 1)
    nc.gpsimd.wait_ge(cc_sem, 2)

    agg_layer(h1rfull, h1own, out_pre, relu_bias=False)

    nc.finalize()
    return nc


def kernel(**inputs):
    from concourse.bass_utils import run_bass_kernel_spmd

    meta, in_maps, aux = _prep(
        inputs["x"], inputs["W1"], inputs["b1"], inputs["W2"], inputs["b2"],
        inputs["edge_index"], inputs["batch"],
    )
    nc = _build(meta)
    res = run_bass_kernel_spmd(nc, in_maps, list(range(N_CORES)))
    pre = np.concatenate(
        [res.results[c]["out_pre"][:NPC] for c in range(N_CORES)], axis=0
    )  # [N_NODES, 256] aggregated pre-W2 layer-2 features
    # host epilogue: mean pool per graph, then @W2 + b2
    counts = aux["counts"]
    sums = np.add.reduceat(pre, aux["seg_starts"], axis=0)
    sums[counts == 0] = 0.0
    pooled = sums / np.maximum(counts, 1)[:, None]
    out = pooled.astype(np.float32) @ aux["W2"] + aux["b2"]
    out[counts == 0] = 0.0
    return out.astype(np.float32)

